# revision 1
# baseline (speedup 1.0000x reference)
"""CTRGC-style GNN message passing block on 8 trn2 NeuronCores.

Sharding: pure data parallel over batch N=256 -> 32 samples/core
(per spec sharding_hint; weights + adjacency A replicated).

Math note: m1/m2 in the reference are means over T of affine projections
of x; mean and affine commute, so we compute xbar = mean_T(x) first and
project the (N,V,C) mean instead of the full (N,V,T,C) tensor — removes
two full-size (N,V,T,R) intermediates with identical results up to fp32
rounding.
"""

import numpy as np
import jax
import jax.numpy as jnp

N, V, T, C_IN, C_OUT, REL, H4, H5 = 256, 25, 128, 64, 64, 9, 18, 6
NCORES = 8


def _forward(x, A, W1, b1, W2, b2, W3, b3, W4a, b4a, W4b, b4b, W5a, b5a, W5b, b5b):
    silu = jax.nn.silu
    xbar = x.mean(axis=2)                          # (n, V, C)
    m1 = xbar @ W1 + b1                            # (n, V, R)
    m2 = xbar @ W2 + b2                            # (n, V, R)
    x3 = x @ W3 + b3                               # (n, V, T, R)
    rel = m1[:, :, None, :] - m2[:, None, :, :]    # (n, V, V, R)
    h = silu(rel @ W4a + b4a)
    h = silu(h @ W4b + b4b)
    h = h + A[None, :, :, None]                    # (n, V, V, R)
    agg = jnp.einsum('nuvc,nvtc->nutc', h, x3)     # (n, V, T, R)
    y = silu(agg @ W5a + b5a)
    y = silu(y @ W5b + b5b)                        # (n, V, T, C_out)
    return y


_PMAPPED = None


def _get_pmapped():
    global _PMAPPED
    if _PMAPPED is None:
        devs = jax.devices()[:NCORES]
        _PMAPPED = jax.pmap(
            _forward, in_axes=(0,) + (None,) * 15, devices=devs
        )
    return _PMAPPED


def kernel(**inputs: np.ndarray) -> np.ndarray:
    x = np.ascontiguousarray(inputs["x"], dtype=np.float32)
    names = ["A", "W1", "b1", "W2", "b2", "W3", "b3",
             "W4a", "b4a", "W4b", "b4b", "W5a", "b5a", "W5b", "b5b"]
    consts = [np.asarray(inputs[k], dtype=np.float32) for k in names]
    xs = x.reshape(NCORES, N // NCORES, V, T, C_IN)
    y = _get_pmapped()(xs, *consts)
    y = np.asarray(y).reshape(N, V, T, C_OUT).astype(np.float32)
    return y

